# revision 19
# baseline (speedup 1.0000x reference)
"""Trainium2 Bass kernel for the 2-layer LSTM decoder with Luong attention.

Strategy: data-parallel over batch (128 -> 16 per core, 8 cores, no cross-core
communication). All weights/context live in SBUF in bf16; activations are kept
"transposed" ([feature, batch]) end-to-end so matmuls are weight-stationary
(lhsT = W.T tile [k,128], rhs = activation [k,16]) and pointwise ops run at
full 128-partition width. The embedding contribution to layer-0 gates
(emb_t @ Wih0[:, :E].T + bias) is precomputed on device as one big matmul
before the recurrent loop. Softmax uses exp without max-subtraction (scores
are ~N(0,8^2); fp32 exp is safe) with the sum accumulated for free via the
activation accum_out port. Only tanh/exp activations are used, which share a
single ACT table set, so there are no per-step table reloads.

Host-side work is restricted to layout prep: batch sharding, embedding row
gather, transposes to the lhsT/rhs layouts, bf16 casts, and gate-order
permutation (i,f,g,o -> i,f,o,g so the sigmoid gates are contiguous).
"""

import os
import sys

sys.path.insert(0, "/opt/trn_rl_repo")

from contextlib import ExitStack

import numpy as np
import ml_dtypes

import concourse.bass as bass
import concourse.tile as tile
from concourse import bacc, mybir
from concourse.bass_utils import run_bass_kernel_spmd
from concourse.masks import make_identity

BF16 = ml_dtypes.bfloat16
F32 = mybir.dt.float32
BF = mybir.dt.bfloat16

L, B, T, S, H, E, V = 2, 128, 64, 80, 512, 512, 32000
NCORES = 8
BC = B // NCORES  # 16 batch rows per core
G4 = 4 * H  # 2048 gate dim
KT = H // 128  # 4 k-chunks per 512 features
MT = G4 // 128  # 16 m-tiles of the gate dim
SPAD = 128  # padded context length for the (b,s) layout
ADD = mybir.AluOpType.add
MULT = mybir.AluOpType.mult


def _bcast(ap, n):
    """Append a stride-0 free dim of size n to an AP (free-dim broadcast)."""
    return bass.AP(tensor=ap.tensor, offset=ap.offset, ap=[*ap.ap, [0, n]])


def _pointwise(nc, wk, st, gs, c_t, h_t, final, ly, hf_out):
    """LSTM cell pointwise. gs [128, MT, BC] f32 with gate tiles ordered
    (i 0:4, f 4:8, o 8:12, g 12:16). Updates c_t (f32) and h_t (bf16)."""
    sig = wk.tile([128, 12, BC], F32, tag="sig")
    # sigmoid(x) = 1 / (1 + exp(-x)); exp and tanh share one ACT table set
    nc.scalar.activation(
        out=sig, in_=gs[:, 0:12, :], func=mybir.ActivationFunctionType.Exp,
        scale=-1.0,
    )
    nc.vector.tensor_scalar_add(sig, sig, 1.0)
    nc.vector.reciprocal(sig, sig)
    gt = wk.tile([128, KT, BC], F32, tag="gt")
    nc.scalar.activation(
        out=gt, in_=gs[:, 12:16, :], func=mybir.ActivationFunctionType.Tanh
    )
    tmp = wk.tile([128, KT, BC], F32, tag="tmp")
    nc.vector.tensor_mul(tmp, sig[:, 0:KT, :], gt)  # i * g
    nc.vector.tensor_mul(c_t, sig[:, KT:2 * KT, :], c_t)  # f * c
    nc.vector.tensor_add(c_t, c_t, tmp)
    th = wk.tile([128, KT, BC], F32, tag="th")
    nc.scalar.activation(
        out=th, in_=c_t, func=mybir.ActivationFunctionType.Tanh
    )
    if final:
        hf = st.tile([128, KT, BC], F32, tag=f"hf{ly}")
        nc.vector.tensor_mul(hf, sig[:, 2 * KT:3 * KT, :], th)
        nc.vector.tensor_copy(h_t, hf)
        hf_out[ly] = hf
    else:
        nc.vector.tensor_mul(h_t, sig[:, 2 * KT:3 * KT, :], th)


def build_program(t_steps=T):
    nc = bacc.Bacc("TRN2", target_bir_lowering=False, debug=False)

    def din(name, shape, dt=BF):
        return nc.dram_tensor(name, list(shape), dt, kind="ExternalInput").ap()

    def dout(name, shape, dt=F32):
        return nc.dram_tensor(name, list(shape), dt, kind="ExternalOutput").ap()

    d = {
        "embT": din("embT", (E, t_steps * BC)),
        "W0eT": din("W0eT", (E, G4)),
        "W0xT": din("W0xT", (H, G4)),
        "Wh0T": din("Wh0T", (H, G4)),
        "W1xT": din("W1xT", (H, G4)),
        "Wh1T": din("Wh1T", (H, G4)),
        "WinT": din("WinT", (H, H)),
        "WoutT": din("WoutT", (2 * H, H)),
        "b0v": din("b0v", (128, MT), F32),
        "b1v": din("b1v", (128, MT), F32),
        "h0T": din("h0T", (L, H, BC)),
        "c0T": din("c0T", (L, H, BC), F32),
        "ctxT": din("ctxT", (H, BC, S)),
        "ctxP": din("ctxP", (SPAD, BC, H)),
    }
    o = {
        "outsT": dout("outsT", (t_steps, H, BC)),
        "attns": dout("attns", (t_steps, S, BC)),
        "hTf": dout("hTf", (L, H, BC)),
        "cTf": dout("cTf", (L, H, BC)),
    }
    hf_out = {}

    with tile.TileContext(nc) as tc, ExitStack() as ctx:
        wp = ctx.enter_context(tc.tile_pool(name="wp", bufs=1))
        st = ctx.enter_context(tc.tile_pool(name="st", bufs=1))
        wk = ctx.enter_context(tc.tile_pool(name="wk", bufs=2))
        gps = ctx.enter_context(tc.tile_pool(name="gps", bufs=2, space="PSUM"))
        sps = ctx.enter_context(tc.tile_pool(name="sps", bufs=2, space="PSUM"))

        def load(name, shape, dt=BF, rearr=None, src=None, tag=None):
            tl = wp.tile(list(shape), dt, tag=tag or name)
            if src is None:
                src = d[name]
            if rearr:
                src = src.rearrange(rearr, p=128)
            nc.sync.dma_start(out=tl, in_=src)
            return tl

        sW0e = load("W0eT", [128, KT, G4], rearr="(kt p) m -> p kt m")
        sW0x = load("W0xT", [128, KT, G4], rearr="(kt p) m -> p kt m")
        sWh0 = load("Wh0T", [128, KT, G4], rearr="(kt p) m -> p kt m")
        sW1x = load("W1xT", [128, KT, G4], rearr="(kt p) m -> p kt m")
        sWh1 = load("Wh1T", [128, KT, G4], rearr="(kt p) m -> p kt m")
        sWin = load("WinT", [128, KT, H], rearr="(kt p) m -> p kt m")
        sWout = load("WoutT", [128, 2 * KT, H], rearr="(kt p) m -> p kt m")
        sB0 = load("b0v", [128, MT], F32)
        sB1 = load("b1v", [128, MT], F32)
        sCtxT = load("ctxT", [128, KT, BC, S], rearr="(kt p) b s -> p kt b s")
        sCtxP = load("ctxP", [128, BC, H])
        sEmb = load("embT", [128, KT, t_steps * BC], rearr="(kt p) n -> p kt n")
        sH, sC = [], []
        for ly in range(L):
            sH.append(load(None, [128, KT, BC], BF, tag=f"h{ly}",
                           rearr="(kt p) b -> p kt b", src=d["h0T"][ly]))
            ctile = st.tile([128, KT, BC], F32, tag=f"c{ly}")
            nc.sync.dma_start(
                out=ctile, in_=d["c0T"][ly].rearrange("(kt p) b -> p kt b", p=128)
            )
            sC.append(ctile)

        sOut = st.tile([128, KT, BC], BF, tag="outprev")
        nc.vector.memset(sOut, 0.0)
        sA = st.tile([128, BC, BC], BF, tag="amat")
        nc.vector.memset(sA, 0.0)
        sOnes = st.tile([128, 1], F32, tag="ones")
        nc.vector.memset(sOnes, 1.0)

        # ---- pre-loop: PG0[g, t, b] = W0e @ embT + b0, stored bf16 ----
        sPG0 = wp.tile([128, MT, t_steps, BC], BF, tag="pg0")
        ncols = min(512, t_steps * BC)
        nch_n = (t_steps * BC) // ncols
        tpb = ncols // BC  # timesteps per psum chunk
        if True:
            for mt in range(MT):
                for nch in range(nch_n):
                    ps = gps.tile([128, ncols], F32, tag="gates")
                    for kt in range(KT):
                        nc.tensor.matmul(
                            ps,
                            lhsT=sW0e[:, kt, mt * 128:(mt + 1) * 128],
                            rhs=sEmb[:, kt, nch * ncols:(nch + 1) * ncols],
                            start=(kt == 0),
                            stop=(kt == KT - 1),
                        )
                    bb = sB0[:, mt:mt + 1]
                    b_bc = bass.AP(
                        tensor=bb.tensor, offset=bb.offset,
                        ap=[bb.ap[0], [0, tpb], [0, BC]],
                    )
                    nc.vector.tensor_add(
                        sPG0[:, mt, nch * tpb:(nch + 1) * tpb, :],
                        ps.rearrange("p (t b) -> p t b", b=BC),
                        b_bc,
                    )

        # ---- recurrent loop ----
        for t in range(t_steps):
            # layer 0 gates: W0x @ out_prev + Wh0 @ h0, then + PG0[t]
            g0 = gps.tile([128, MT, BC], F32, tag="gates")
            for mt in range(MT):
                for kt in range(KT):
                    nc.tensor.matmul(
                        g0[:, mt, :],
                        lhsT=sW0x[:, kt, mt * 128:(mt + 1) * 128],
                        rhs=sOut[:, kt, :],
                        start=(kt == 0),
                        stop=False,
                    )
                for kt in range(KT):
                    nc.tensor.matmul(
                        g0[:, mt, :],
                        lhsT=sWh0[:, kt, mt * 128:(mt + 1) * 128],
                        rhs=sH[0][:, kt, :],
                        start=False,
                        stop=(kt == KT - 1),
                    )
            gs0 = wk.tile([128, MT, BC], F32, tag="gs")
            nc.vector.tensor_add(gs0, g0, sPG0[:, :, t, :])
            _pointwise(nc, wk, st, gs0, sC[0], sH[0],
                       final=(t == t_steps - 1), ly=0, hf_out=hf_out)

            # layer 1 gates
            g1 = gps.tile([128, MT, BC], F32, tag="gates")
            for mt in range(MT):
                for kt in range(KT):
                    nc.tensor.matmul(
                        g1[:, mt, :],
                        lhsT=sW1x[:, kt, mt * 128:(mt + 1) * 128],
                        rhs=sH[0][:, kt, :],
                        start=(kt == 0),
                        stop=False,
                    )
                for kt in range(KT):
                    nc.tensor.matmul(
                        g1[:, mt, :],
                        lhsT=sWh1[:, kt, mt * 128:(mt + 1) * 128],
                        rhs=sH[1][:, kt, :],
                        start=False,
                        stop=(kt == KT - 1),
                    )
            gs1 = wk.tile([128, MT, BC], F32, tag="gs")
            nc.vector.tensor_add(gs1, g1, _bcast(sB1, BC))
            _pointwise(nc, wk, st, gs1, sC[1], sH[1],
                       final=(t == t_steps - 1), ly=1, hf_out=hf_out)

            # q = Win @ h1 -> bf16
            qp = sps.tile([128, KT, BC], F32, tag="ps1")
            for mq in range(KT):
                for kt in range(KT):
                    nc.tensor.matmul(
                        qp[:, mq, :],
                        lhsT=sWin[:, kt, mq * 128:(mq + 1) * 128],
                        rhs=sH[1][:, kt, :],
                        start=(kt == 0),
                        stop=(kt == KT - 1),
                    )
            qsb = wk.tile([128, KT, BC], BF, tag="qsb")
            nc.vector.tensor_copy(qsb, qp)

            # scoresT[s, b] = sum_h ctxT[h,b,s] * q[h,b]  (ctx stationary)
            sc = sps.tile([S, BC], F32, tag="ps2")
            for b in range(BC):
                for kt in range(KT):
                    nc.tensor.matmul(
                        sc[:, b:b + 1],
                        lhsT=sCtxT[:, kt, b, :],
                        rhs=qsb[:, kt, b:b + 1],
                        start=(kt == 0),
                        stop=(kt == KT - 1),
                    )
            # softmax in [s, b] layout, no max subtraction (scores are O(30))
            expT = wk.tile([S, BC], F32, tag="expT")
            nc.scalar.activation(
                out=expT, in_=sc, func=mybir.ActivationFunctionType.Exp
            )
            # column sums via ones.T @ expT -> [1, BC]
            esum = sps.tile([1, BC], F32, tag="ps1")
            nc.tensor.matmul(esum, lhsT=sOnes[:S, :], rhs=expT,
                             start=True, stop=True)
            erec = wk.tile([1, BC], F32, tag="erec")
            nc.vector.reciprocal(erec, esum)
            # broadcast recip across s rows: R[s,b] = recip[b] via outer
            # product with a k=1 ones row
            rbc = sps.tile([S, BC], F32, tag="ps2")
            ones_row = bass.AP(
                tensor=sOnes.tensor, offset=sOnes.offset,
                ap=[[sOnes.ap[0][0], 1], [0, S]],
            )
            nc.tensor.matmul(rbc, lhsT=ones_row, rhs=erec,
                             start=True, stop=True)
            # attnT = expT * R; bf16 copy onto the block-diagonal of A
            attnT = wk.tile([S, BC], F32, tag="attnT")
            nc.vector.tensor_mul(attnT, expT, rbc)
            adiag = bass.AP(
                tensor=sA.tensor, offset=sA.offset,
                ap=[[sA.ap[0][0], S], [BC + 1, BC]],
            )
            nc.vector.tensor_copy(adiag, attnT)
            nc.sync.dma_start(out=o["attns"][t], in_=attnT)

            # wctxT[h, b] = sum_{b',s} ctxP[(b',s), h] * A[(b',s), b]
            wps = sps.tile([128, KT, BC], F32, tag="ps1")
            for mt in range(KT):
                for b in range(BC):
                    nc.tensor.matmul(
                        wps[:, mt, :],
                        lhsT=sCtxP[:, b, mt * 128:(mt + 1) * 128],
                        rhs=sA[:, b, :],
                        start=(b == 0),
                        stop=(b == BC - 1),
                    )
            wsb = wk.tile([128, KT, BC], BF, tag="wsb")
            nc.vector.tensor_copy(wsb, wps)

            # out = tanh(Wout @ [wctx; h1])
            op_ = sps.tile([128, KT, BC], F32, tag="ps2")
            for mo in range(KT):
                for kt in range(KT):
                    nc.tensor.matmul(
                        op_[:, mo, :],
                        lhsT=sWout[:, kt, mo * 128:(mo + 1) * 128],
                        rhs=wsb[:, kt, :],
                        start=(kt == 0),
                        stop=False,
                    )
                for kt in range(KT):
                    nc.tensor.matmul(
                        op_[:, mo, :],
                        lhsT=sWout[:, KT + kt, mo * 128:(mo + 1) * 128],
                        rhs=sH[1][:, kt, :],
                        start=False,
                        stop=(kt == KT - 1),
                    )
            outF = wk.tile([128, KT, BC], F32, tag="outF")
            nc.scalar.activation(
                out=outF, in_=op_, func=mybir.ActivationFunctionType.Tanh
            )
            nc.vector.tensor_copy(sOut, outF)
            nc.sync.dma_start(
                out=o["outsT"][t].rearrange("(kt p) b -> p kt b", p=128),
                in_=outF,
            )

        # final states
        for ly in range(L):
            nc.sync.dma_start(
                out=o["cTf"][ly].rearrange("(kt p) b -> p kt b", p=128),
                in_=sC[ly],
            )
            nc.sync.dma_start(
                out=o["hTf"][ly].rearrange("(kt p) b -> p kt b", p=128),
                in_=hf_out[ly],
            )

    nc.compile()
    return nc


# ---------------------------------------------------------------------------
# host side
# ---------------------------------------------------------------------------

_PROG_CACHE = {}


def _get_program(t_steps):
    if t_steps not in _PROG_CACHE:
        _PROG_CACHE[t_steps] = build_program(t_steps)
    return _PROG_CACHE[t_steps]


_RUNNER_CACHE = {}


def _get_runner(t_steps):
    """Cached jitted SPMD runner (mirrors bass2jax.run_bass_via_pjrt's
    multi-core path, but the jax.jit object persists across calls)."""
    if t_steps in _RUNNER_CACHE:
        return _RUNNER_CACHE[t_steps]
    import jax
    import jax.numpy as jnp  # noqa: F401
    from jax.experimental.shard_map import shard_map
    from jax.sharding import Mesh, PartitionSpec
    from concourse import bass2jax, mybir as _mybir

    nc = _get_program(t_steps)
    bass2jax.install_neuronx_cc_hook()
    partition_name = (
        nc.partition_id_tensor.name if nc.partition_id_tensor else None
    )
    in_names, out_names, out_avals, zero_shapes = [], [], [], []
    for alloc in nc.m.functions[0].allocations:
        if not isinstance(alloc, _mybir.MemoryLocationSet):
            continue
        name = alloc.memorylocations[0].name
        if alloc.kind == "ExternalInput":
            if name != partition_name:
                in_names.append(name)
        elif alloc.kind == "ExternalOutput":
            shape = tuple(alloc.tensor_shape)
            dtype = _mybir.dt.np(alloc.dtype)
            out_names.append(name)
            out_avals.append(jax.core.ShapedArray(shape, dtype))
            zero_shapes.append((shape, dtype))
    n_params = len(in_names)
    n_outs = len(out_names)
    all_in_names = list(in_names) + list(out_names)
    if partition_name is not None:
        all_in_names = all_in_names + [partition_name]

    def _body(*args):
        operands = list(args)
        if partition_name is not None:
            operands.append(bass2jax.partition_id_tensor())
        outs = bass2jax._bass_exec_p.bind(
            *operands,
            out_avals=tuple(out_avals),
            in_names=tuple(all_in_names),
            out_names=tuple(out_names),
            lowering_input_output_aliases=(),
            sim_require_finite=True,
            sim_require_nnan=True,
            nc=nc,
        )
        return tuple(outs)

    devices = jax.devices()[:NCORES]
    mesh = Mesh(np.array(devices), ("core",))
    in_specs = (PartitionSpec("core"),) * (n_params + n_outs)
    out_specs = (PartitionSpec("core"),) * n_outs
    donate = tuple(range(n_params, n_params + n_outs))
    sharded = jax.jit(
        shard_map(_body, mesh=mesh, in_specs=in_specs, out_specs=out_specs,
                  check_rep=False),
        donate_argnums=donate, keep_unused=True,
    )
    runner = {
        "sharded": sharded, "in_names": in_names, "out_names": out_names,
        "zero_shapes": zero_shapes, "out_avals": out_avals,
    }
    _RUNNER_CACHE[t_steps] = runner
    return runner


def _run_spmd(in_maps, t_steps, time_exec=False):
    """Run the SPMD program; returns (list of per-core out dicts, exec_sec)."""
    import jax

    r = _get_runner(t_steps)
    concat_in = [
        np.concatenate([np.asarray(m[name]) for m in in_maps], axis=0)
        for name in r["in_names"]
    ]
    zeros = [np.zeros((NCORES * s[0], *s[1:]), dt)
             for (s, dt) in r["zero_shapes"]]
    out = r["sharded"](*concat_in, *zeros)
    jax.block_until_ready(out)
    exec_sec = None
    if time_exec:
        dev_in = jax.device_put(concat_in)
        jax.block_until_ready(dev_in)
        times = []
        import time as _time
        for _ in range(3):
            zeros = [np.zeros((NCORES * s[0], *s[1:]), dt)
                     for (s, dt) in r["zero_shapes"]]
            t0 = _time.time()
            out = r["sharded"](*dev_in, *zeros)
            jax.block_until_ready(out)
            times.append(_time.time() - t0)
        exec_sec = min(times)
    results = []
    for c in range(NCORES):
        results.append({
            name: np.asarray(out[i]).reshape(NCORES, *r["out_avals"][i].shape)[c]
            for i, name in enumerate(r["out_names"])
        })
    return results, exec_sec


def _prep_core_inputs(inputs, c, t_steps):
    """Build the per-core input map (batch columns c*BC:(c+1)*BC)."""
    f32 = np.float32
    cols = slice(c * BC, (c + 1) * BC)
    tok = np.asarray(inputs["tokens"])[:t_steps, cols]
    emb_table = np.asarray(inputs["emb_table"], f32)
    emb = emb_table[tok]  # [t, BC, E]
    embT = np.ascontiguousarray(emb.transpose(2, 0, 1).reshape(E, t_steps * BC))

    perm = np.concatenate([
        np.arange(0, H), np.arange(H, 2 * H),
        np.arange(3 * H, 4 * H), np.arange(2 * H, 3 * H),
    ])
    Wih0 = np.asarray(inputs["Wih0"], f32)[perm]
    Whh0 = np.asarray(inputs["Whh0"], f32)[perm]
    Wih1 = np.asarray(inputs["Wih1"], f32)[perm]
    Whh1 = np.asarray(inputs["Whh1"], f32)[perm]
    b0 = (np.asarray(inputs["bih0"], f32) + np.asarray(inputs["bhh0"], f32))[perm]
    b1 = (np.asarray(inputs["bih1"], f32) + np.asarray(inputs["bhh1"], f32))[perm]

    h0 = np.asarray(inputs["h0"], f32)[:, cols, :]  # [L, BC, H]
    c0 = np.asarray(inputs["c0"], f32)[:, cols, :]
    ctx = np.asarray(inputs["context"], f32)[cols]  # [BC, S, H]
    ctxP = np.zeros((SPAD, BC, H), f32)
    ctxP[:S] = ctx.transpose(1, 0, 2)

    def bfc(x):
        return np.ascontiguousarray(x).astype(BF16)

    return {
        "embT": bfc(embT),
        "W0eT": bfc(Wih0[:, :E].T),
        "W0xT": bfc(Wih0[:, E:].T),
        "Wh0T": bfc(Whh0.T),
        "W1xT": bfc(Wih1.T),
        "Wh1T": bfc(Whh1.T),
        "WinT": bfc(np.asarray(inputs["W_in"], f32).T),
        "WoutT": bfc(np.asarray(inputs["W_out"], f32).T),
        "b0v": np.ascontiguousarray(b0.reshape(MT, 128).T),
        "b1v": np.ascontiguousarray(b1.reshape(MT, 128).T),
        "h0T": bfc(h0.transpose(0, 2, 1)),
        "c0T": np.ascontiguousarray(c0.transpose(0, 2, 1)),
        "ctxT": bfc(ctx.transpose(2, 0, 1)),
        "ctxP": bfc(ctxP),
    }


def kernel(**inputs):
    return _kernel_t(inputs, T)


LAST_EXEC_SEC = None


def _kernel_t(inputs, t_steps, time_exec=False):
    global LAST_EXEC_SEC
    import time as _time
    t0 = _time.time()
    in_maps = [_prep_core_inputs(inputs, c, t_steps) for c in range(NCORES)]
    t1 = _time.time()
    results, exec_sec = _run_spmd(in_maps, t_steps, time_exec=time_exec)
    t2 = _time.time()
    LAST_EXEC_SEC = exec_sec
    if os.environ.get("DEC_TIMING"):
        print("  [prep %.3fs, spmd %.3fs, exec %s]" % (
            t1 - t0, t2 - t1,
            "%.4fs" % exec_sec if exec_sec else "n/a"))
    f32 = np.float32
    outputs = np.empty((t_steps, B, H), f32)
    attns = np.empty((t_steps, B, S), f32)
    hT = np.empty((L, B, H), f32)
    cT = np.empty((L, B, H), f32)
    for c in range(NCORES):
        r = results[c]
        cols = slice(c * BC, (c + 1) * BC)
        outputs[:, cols, :] = np.asarray(r["outsT"]).transpose(0, 2, 1)
        attns[:, cols, :] = np.asarray(r["attns"]).transpose(0, 2, 1)
        hT[:, cols, :] = np.asarray(r["hTf"]).transpose(0, 2, 1)
        cT[:, cols, :] = np.asarray(r["cTf"]).transpose(0, 2, 1)
    return outputs, (hT, cT), attns
